# revision 22
# baseline (speedup 1.0000x reference)
"""Trainium2 Bass kernel for nn_CXINGeneral_1425929142863 (GNN message passing).

Math (per branch b, epsilon=0):
    agg_b = A_b @ x_src_b               (gather + segment-sum, IN_CH=128 space)
    h_b   = relu-MLP_b( agg_b @ W_b + x_target )      (3 layers)
    out   = concat(h0, h1) @ Wm + bm

Key rewrite: A @ (x_src @ W) == (A @ x_src) @ W — aggregate in IN_CH space,
making every dense matmul local to the target shard. Target rows sharded 8
ways; all weights replicated; no collectives.

v13 layout (vs v12's DMA'd one-hot stream):
  - The scatter one-hot S is built ON DEVICE: per edge we ship only a
    bf16 block-local row index d (2 B/edge); S = is_equal(iota, d) via one
    broadcast DVE op per window/branch. Saves ~28 MB/core of HBM reads.
  - Narrow scatter slots (28 rows instead of 128). Matmul cost on TensorE is
    out-width cycles, so scatter PE time drops 4x. To avoid ceil-padding
    blowup at small widths, target rows are PERMUTED host-side: a greedy
    min-max bin packer balances per-slot edge counts across BOTH branches so
    every 28-row slot needs exactly ceil(<=256/128)=2 chunks of 128 edges.
    The host inverse-permutes the output rows at the end.
  - Output written bf16 (host upcasts; ~0.1% rms, well inside tolerance).

Per window (512 rows = 18 slots x 28 + 1 slot x 8; last window 3x28+22):
  scatter chunks accumulate X_chunk.T @ S_chunk into PSUM [128ch, rows];
  dense (transposed activations [ch, rows]): head matmul + x_target add,
  3x (matmul + relu on ACT); merge: 4 accumulating matmuls + bias add.
Branch 0/1 work is interleaved so TensorE never waits on ACT/DVE.
"""

import heapq
import sys
import types

import numpy as np
import ml_dtypes

import concourse.bass as bass
import concourse.mybir as mybir
import concourse.tile as tile
from concourse import bacc
import concourse.bass_utils as bass_utils
from concourse.bass_utils import run_bass_kernel_spmd

F32 = mybir.dt.float32
BF16 = mybir.dt.bfloat16
I32 = mybir.dt.int32
BF16_NP = ml_dtypes.bfloat16


def _install_profile_hook():
    """This container's antenv lacks axon_hooks; reconstruct so trace=True works."""
    try:
        import antenv.axon_hooks  # noqa: F401
        return
    except ImportError:
        pass
    try:
        from trn_agent_boot.trn_boot import _ntff_profile_via_ctypes
    except ImportError:
        return
    mod = types.ModuleType("antenv.axon_hooks")
    hook = _ntff_profile_via_ctypes("/opt/axon/libaxon_pjrt.so")
    mod.get_axon_ntff_profile_hook = lambda: hook
    sys.modules["antenv.axon_hooks"] = mod
    bass_utils.upload_artifacts = lambda tmpdir: f"local:{tmpdir}"


class Cfg:
    def __init__(self):
        self.N_T = 50000
        self.N_S = 100000
        self.E = 400000
        self.NC = 8
        self.IN_CH = 128
        self.OUT_CH = 256
        self.N_MLP = 3
        self.NT_LOC = self.N_T // self.NC      # 6250
        self.WIN = 512
        self.SW = 28                            # big slot width
        # per-core grid: 12 windows of (18x28 + 1x8), 1 window of (3x28 + 1x22)
        self.grid = []
        for _ in range(12):
            self.grid.append([(28, i * 28) for i in range(18)] + [(8, 504)])
        self.grid.append([(28, i * 28) for i in range(3)] + [(22, 84)])
        self.n_wins = len(self.grid)            # 13
        self.slots_per_win = [len(w) for w in self.grid]
        self.n_gslots = sum(self.slots_per_win)  # 232
        # gslot id = running index in (window, slot) order
        self.gslot_base = np.cumsum([0] + self.slots_per_win).astype(np.int64)


CFG = Cfg()


# ----------------------------------------------------------------- host prep

def _balance(cfg, deg0, deg1):
    """Permute target rows so each (core, window, slot) bin has balanced edge
    counts in BOTH branches. Returns perm (new->old) and per-bin row lists
    keyed by (core, win, slot)."""
    total = deg0 + deg1
    order_asc = np.argsort(total, kind="stable")

    n_small8 = cfg.NC * 12          # 96 bins of 8 rows
    n_tail22 = cfg.NC               # 8 bins of 22 rows
    n_small_rows = n_small8 * 8 + n_tail22 * 22   # 944
    small_rows = order_asc[:n_small_rows]
    big_rows = order_asc[n_small_rows:][::-1]     # descending degree

    # --- greedy min-max packing into 28-row bins
    n_big = cfg.NC * (12 * 18 + 3)  # 1752
    heap = [(0, 0, 0, 0, b) for b in range(n_big)]
    heapq.heapify(heap)
    assign = [[] for _ in range(n_big)]
    loads = [(0, 0)] * n_big
    d0l = deg0[big_rows]
    d1l = deg1[big_rows]
    for i in range(len(big_rows)):
        r = big_rows[i]
        _, l0, l1, n, b = heapq.heappop(heap)
        assign[b].append(r)
        l0 += int(d0l[i])
        l1 += int(d1l[i])
        n += 1
        loads[b] = (l0, l1)
        if n < 28:
            heapq.heappush(heap, (max(l0, l1), l0, l1, n, b))

    # heavy bins grouped 8-at-a-time onto the same (win, slot) across cores
    bin_order = sorted(range(n_big), key=lambda b: -max(*loads[b]))
    big_positions = [(w, s) for w in range(12) for s in range(18)] + \
                    [(12, s) for s in range(3)]
    bins = {}
    for g, (w, s) in enumerate(big_positions):
        for c in range(cfg.NC):
            bins[(c, w, s)] = assign[bin_order[g * cfg.NC + c]]

    # small bins: lowest-degree rows, sequential fill
    p = 0
    for i in range(n_small8):
        c, w = i % cfg.NC, i // cfg.NC
        bins[(c, w, 18)] = list(small_rows[p:p + 8])
        p += 8
    for c in range(cfg.NC):
        bins[(c, 12, 3)] = list(small_rows[p:p + 22])
        p += 22

    perm = np.empty(cfg.N_T, np.int64)
    for c in range(cfg.NC):
        base = c * cfg.NT_LOC
        for w in range(cfg.n_wins):
            for s, (wd, roff) in enumerate(cfg.grid[w]):
                rows = bins[(c, w, s)]
                assert len(rows) == wd
                perm[base + w * cfg.WIN + roff:
                     base + w * cfg.WIN + roff + wd] = rows
    return perm


def _edge_fields(cfg, rows_new):
    """Map permuted row index -> (core, win, gslot, local_d)."""
    core = rows_new // cfg.NT_LOC
    lrow = rows_new % cfg.NT_LOC
    win = lrow // cfg.WIN
    wrow = lrow % cfg.WIN
    n28 = np.where(win < 12, 18, 3)
    s_idx = np.minimum(wrow // cfg.SW, n28)
    roff = np.where(s_idx < n28, s_idx * cfg.SW, n28 * cfg.SW)
    local_d = wrow - roff
    gslot = cfg.gslot_base[win] + s_idx
    return core, win, gslot, local_d


def _structure(cfg, k):
    """Derive chunk-stream layout from per-gslot chunk counts k (len 232).

    Returns dict with per-window chunk descriptor lists and bases."""
    k = np.asarray(k, np.int64)
    st = {"k": k}
    chunk_base = np.zeros(cfg.n_gslots, np.int64)
    np.cumsum(k[:-1], out=chunk_base[1:])
    st["chunk_base"] = chunk_base
    st["K_total"] = int(k.sum())
    win_chunks = []     # per window: list of (psc_off, width, start, stop)
    win_base = []       # first chunk col of window
    n28_l, ksm_l, smw_l, swidth_l = [], [], [], []
    for w in range(cfg.n_wins):
        g0 = cfg.gslot_base[w]
        chunks = []
        scol = 0
        n28 = 0
        for s, (wd, roff) in enumerate(cfg.grid[w]):
            kk = int(k[g0 + s])
            for i in range(kk):
                chunks.append((roff, wd, i == 0, i == kk - 1))
            if wd == cfg.SW:
                n28 += kk
            else:
                ksm_l.append(kk)
                smw_l.append(wd)
            scol += kk * wd
        win_chunks.append(chunks)
        win_base.append(int(chunk_base[g0]))
        n28_l.append(n28)
        swidth_l.append(scol)
    st["win_chunks"] = win_chunks
    st["win_base"] = win_base
    st["n28"] = n28_l
    st["ksm"] = ksm_l          # per window small-slot chunk count
    st["smw"] = smw_l          # per window small-slot width (8 or 22)
    st["s_width"] = swidth_l   # one-hot tile cols per window
    st["Kwin_max"] = max(len(c) for c in win_chunks)
    st["swidth_max"] = max(swidth_l)
    st["n28_max"] = max(n28_l)
    st["k8_max"] = max(ksm_l[w] for w in range(12))
    st["k22_max"] = ksm_l[12]
    return st


def prep_inputs(cfg, inputs):
    deg0 = np.bincount(np.asarray(inputs["rows0"], np.int64), minlength=cfg.N_T)
    deg1 = np.bincount(np.asarray(inputs["rows1"], np.int64), minlength=cfg.N_T)
    perm = _balance(cfg, deg0, deg1)
    pos = np.empty(cfg.N_T, np.int64)
    pos[perm] = np.arange(cfg.N_T)

    # per-branch edge fields + per-gslot counts (max over cores+branches)
    br_fields = []
    counts = np.zeros((2, cfg.NC, cfg.n_gslots), np.int64)
    for b in (0, 1):
        rows_new = pos[np.asarray(inputs[f"rows{b}"], np.int64)]
        core, win, gslot, local_d = _edge_fields(cfg, rows_new)
        np.add.at(counts[b], (core, gslot), 1)
        br_fields.append((core, gslot, local_d))
    k = np.maximum(1, -(-counts.max(axis=(0, 1)) // 128))
    st = _structure(cfg, k)
    K = st["K_total"]
    chunk_base = st["chunk_base"]

    # pack x / d streams
    xd = {}
    for b in (0, 1):
        core, gslot, local_d = br_fields[b]
        cols = np.asarray(inputs[f"cols{b}"], np.int64)
        vals = np.asarray(inputs[f"vals{b}"], np.float32)
        xsrc = np.asarray(inputs[f"x_src{b}"], np.float32)
        key = core * cfg.n_gslots + gslot
        order = np.argsort(key, kind="stable")
        key_s = key[order]
        grp_counts = np.bincount(key_s, minlength=cfg.NC * cfg.n_gslots)
        starts = np.zeros(cfg.NC * cfg.n_gslots, np.int64)
        np.cumsum(grp_counts[:-1], out=starts[1:])
        rank = np.arange(len(key)) - starts[key_s]
        core_s = core[order]
        chunkcol = chunk_base[gslot[order]] + rank // 128
        lane = rank % 128
        x_arr = np.zeros((cfg.NC, 128, K, 128), BF16_NP)
        d_arr = np.zeros((cfg.NC, 128, K), BF16_NP)
        x_arr[core_s, lane, chunkcol] = \
            (vals[order][:, None] * xsrc[cols[order]]).astype(BF16_NP)
        d_arr[core_s, lane, chunkcol] = local_d[order].astype(BF16_NP)
        xd[f"x{b}"] = x_arr
        xd[f"d{b}"] = d_arr

    zero_bias = (not np.any(np.asarray(inputs["mlp_b0"]))
                 and not np.any(np.asarray(inputs["mlp_b1"])))

    # ---- weights (same layouts as v12)
    x_target = np.asarray(inputs["x_target"], np.float32)
    W0 = np.asarray(inputs["W0"], np.float32)
    W1 = np.asarray(inputs["W1"], np.float32)
    w01 = np.ascontiguousarray(np.concatenate([W0, W1], axis=1)).astype(BF16_NP)

    mlpw = []
    for b in (0, 1):
        mw = np.asarray(inputs[f"mlp_W{b}"], np.float32)
        blocks = []
        for l in range(cfg.N_MLP):
            for icb in range(2):
                for ocb in range(2):
                    blocks.append(mw[l, icb * 128:(icb + 1) * 128,
                                     ocb * 128:(ocb + 1) * 128])
        mlpw.append(np.ascontiguousarray(
            np.concatenate(blocks, axis=1)).astype(BF16_NP))

    mlpb = []
    for b in (0, 1):
        mb_ = np.asarray(inputs[f"mlp_b{b}"], np.float32)
        cols_ = []
        for l in range(cfg.N_MLP):
            for ocb in range(2):
                cols_.append(mb_[l, ocb * 128:(ocb + 1) * 128][:, None])
        mlpb.append(np.ascontiguousarray(np.concatenate(cols_, axis=1)))

    Wm = np.asarray(inputs["Wm"], np.float32)
    wm = np.ascontiguousarray(
        np.concatenate([Wm[i * 128:(i + 1) * 128, :] for i in range(4)], axis=1)
    ).astype(BF16_NP)
    bmt = np.ascontiguousarray(
        np.tile(np.asarray(inputs["bm"], np.float32), (128, 2)))

    in_maps = []
    for c in range(cfg.NC):
        xt_loc = x_target[perm[c * cfg.NT_LOC:(c + 1) * cfg.NT_LOC]]
        xt = np.zeros((128, cfg.n_wins * 2 * cfg.WIN), BF16_NP)
        for wi in range(cfg.n_wins):
            w0 = wi * cfg.WIN
            wl = min(cfg.WIN, cfg.NT_LOC - w0)
            for sub in range(2):
                s0 = sub * 256
                if s0 >= wl:
                    break
                sl = min(256, wl - s0)
                for ocb in range(2):
                    base = wi * 2 * cfg.WIN + sub * 512 + ocb * 256
                    xt[:, base:base + sl] = \
                        xt_loc[w0 + s0:w0 + s0 + sl,
                               ocb * 128:(ocb + 1) * 128].T
        m = {
            "xt": xt,
            "w01": w01, "mlpw0": mlpw[0], "mlpw1": mlpw[1],
            "b0": mlpb[0], "b1": mlpb[1],
            "wm": wm, "bmt": bmt,
        }
        for b in (0, 1):
            m[f"x{b}"] = np.ascontiguousarray(xd[f"x{b}"][c]).reshape(128, K * 128)
            m[f"d{b}"] = np.ascontiguousarray(xd[f"d{b}"][c])
        in_maps.append(m)
    return in_maps, st, zero_bias, perm


# ------------------------------------------------------------------- builder

def build(cfg, st, zero_bias):
    nc = bacc.Bacc("TRN2", target_bir_lowering=False, debug=False)

    K = st["K_total"]
    n_wins = cfg.n_wins
    x_d = [nc.declare_dram_parameter(f"x{b}", [128, K * 128], BF16, isOutput=False)
           for b in (0, 1)]
    d_d = [nc.declare_dram_parameter(f"d{b}", [128, K], BF16, isOutput=False)
           for b in (0, 1)]
    xt_d = nc.declare_dram_parameter("xt", [128, n_wins * 2 * cfg.WIN], BF16,
                                     isOutput=False)
    w01_d = nc.declare_dram_parameter("w01", [128, 512], BF16, isOutput=False)
    mlpw_d = [nc.declare_dram_parameter(f"mlpw{b}", [128, cfg.N_MLP * 4 * 128], BF16,
                                        isOutput=False) for b in (0, 1)]
    b_d = [nc.declare_dram_parameter(f"b{b}", [128, cfg.N_MLP * 2], F32, isOutput=False)
           for b in (0, 1)]
    wm_d = nc.declare_dram_parameter("wm", [128, 4 * cfg.OUT_CH], BF16, isOutput=False)
    bmt_d = nc.declare_dram_parameter("bmt", [128, 2 * cfg.OUT_CH], F32, isOutput=False)
    out_d = nc.declare_dram_parameter("out", [cfg.NT_LOC, cfg.OUT_CH], BF16,
                                      isOutput=True)

    wins = []
    w0 = 0
    while w0 < cfg.NT_LOC:
        wins.append((w0, min(cfg.WIN, cfg.NT_LOC - w0)))
        w0 += cfg.WIN

    RELU = mybir.ActivationFunctionType.Relu
    EQ = mybir.AluOpType.is_equal

    with tile.TileContext(nc) as tc:
        with (
            tc.tile_pool(name="wpool", bufs=1) as wpool,
            tc.tile_pool(name="xwin", bufs=3) as xpool,
            tc.tile_pool(name="xtwin", bufs=3) as xtpool,
            tc.tile_pool(name="swin", bufs=2) as spool,
            tc.tile_pool(name="aggp", bufs=2) as aggp,
            tc.tile_pool(name="hwin", bufs=2) as hwin,
            tc.tile_pool(name="hfin", bufs=2) as hfin,
            tc.tile_pool(name="outp", bufs=2) as outp,
            tc.tile_pool(name="pscat", bufs=2, space="PSUM") as pscat,
            tc.tile_pool(name="pdense", bufs=4, space="PSUM") as pdense,
            tc.tile_pool(name="pmerge", bufs=2, space="PSUM") as pmerge,
        ):
            # --- per-edge index streams first: the prologue scatter needs
            # only d + the first window's x, so keep big weight DMAs behind.
            d_sb = []
            for b in (0, 1):
                td = wpool.tile([128, K], BF16, tag=f"d{b}")
                nc.sync.dma_start(out=td[:], in_=d_d[b][:])
                d_sb.append(td)

            # --- iota tiles for on-device one-hot build
            io28 = wpool.tile([128, st["n28_max"] * 28], BF16, tag="io28")
            nc.gpsimd.iota(out=io28[:], pattern=[[0, st["n28_max"]], [1, 28]],
                           base=0, channel_multiplier=0,
                           allow_small_or_imprecise_dtypes=True)
            io8 = wpool.tile([128, st["k8_max"] * 8], BF16, tag="io8")
            nc.gpsimd.iota(out=io8[:], pattern=[[0, st["k8_max"]], [1, 8]],
                           base=0, channel_multiplier=0,
                           allow_small_or_imprecise_dtypes=True)
            io22 = wpool.tile([128, st["k22_max"] * 22], BF16, tag="io22")
            nc.gpsimd.iota(out=io22[:], pattern=[[0, st["k22_max"]], [1, 22]],
                           base=0, channel_multiplier=0,
                           allow_small_or_imprecise_dtypes=True)

            def load_win(wi):
                """Prefetch window wi's x chunk streams (both branches, split
                into two half-DMAs so scatter can start on the first half)
                and its x_target slice."""
                wb = st["win_base"][wi]
                kw = len(st["win_chunks"][wi])
                half = kw // 2
                tiles = []
                for br in (0, 1):
                    xw = xpool.tile([128, st["Kwin_max"] * 128], BF16, tag=f"xw{br}")
                    xh = [xw[:, :half * 128], xw[:, half * 128:kw * 128]]
                    nc.sync.dma_start(
                        out=xh[0],
                        in_=x_d[br][:, wb * 128:(wb + half) * 128])
                    nc.sync.dma_start(
                        out=xh[1],
                        in_=x_d[br][:, (wb + half) * 128:(wb + kw) * 128])
                    tiles.append(xw)
                xt_w = xtpool.tile([128, 2 * cfg.WIN], BF16, tag="xt")
                nc.sync.dma_start(
                    out=xt_w[:],
                    in_=xt_d[:, wi * 2 * cfg.WIN:(wi + 1) * 2 * cfg.WIN])
                tiles.append(xt_w)
                return tiles

            def emit_onehot(wi, br):
                """Build window wi's one-hot scatter tile for branch br."""
                wb = st["win_base"][wi]
                n28 = st["n28"][wi]
                ksm = st["ksm"][wi]
                smw = st["smw"][wi]
                s = spool.tile([128, st["swidth_max"]], BF16, tag=f"s{br}")
                nc.vector.tensor_tensor(
                    out=s[:, :n28 * 28].rearrange("p (n w) -> p n w", w=28),
                    in0=io28[:, :n28 * 28].rearrange("p (n w) -> p n w", w=28),
                    in1=d_sb[br][:, wb:wb + n28].unsqueeze(2)
                        .broadcast_to([128, n28, 28]),
                    op=EQ)
                off = n28 * 28
                iot = io8 if smw == 8 else io22
                nc.vector.tensor_tensor(
                    out=s[:, off:off + ksm * smw]
                        .rearrange("p (n w) -> p n w", w=smw),
                    in0=iot[:, :ksm * smw].rearrange("p (n w) -> p n w", w=smw),
                    in1=d_sb[br][:, wb + n28:wb + n28 + ksm].unsqueeze(2)
                        .broadcast_to([128, ksm, smw]),
                    op=EQ)
                return s

            # process the short (106-row) window FIRST: its x stream is tiny
            # (fast pipeline fill while weights stream in) and the final
            # window then has 4 full dependency chains (no 2-chain tail).
            worder = [len(wins) - 1] + list(range(len(wins) - 1))
            win_tiles = {worder[0]: load_win(worder[0])}

            # weights ride the scalar queue so they never block the x stream
            w01_sb = wpool.tile([128, 512], BF16, tag="w01")
            nc.scalar.dma_start(out=w01_sb[:], in_=w01_d[:])
            win_tiles[worder[1]] = load_win(worder[1])
            mlpw_sb, b_sb = [], []
            for b in (0, 1):
                t = wpool.tile([128, cfg.N_MLP * 4 * 128], BF16, tag=f"mlpw{b}")
                for l in range(cfg.N_MLP):
                    nc.scalar.dma_start(
                        out=t[:, l * 512:(l + 1) * 512],
                        in_=mlpw_d[b][:, l * 512:(l + 1) * 512])
                mlpw_sb.append(t)
                if not zero_bias:
                    tb = wpool.tile([128, cfg.N_MLP * 2], F32, tag=f"bias{b}")
                    nc.scalar.dma_start(out=tb[:], in_=b_d[b][:])
                    b_sb.append(tb)
            wm_sb = wpool.tile([128, 4 * cfg.OUT_CH], BF16, tag="wm")
            nc.scalar.dma_start(out=wm_sb[:], in_=wm_d[:])
            bmt_sb = wpool.tile([128, 2 * cfg.OUT_CH], F32, tag="bmt")
            nc.scalar.dma_start(out=bmt_sb[:], in_=bmt_d[:])

            hfin_store = {}
            sc_state = {}
            aggw_store = {}
            out_rr = [0]

            def emit_sc_half(wi, br, half):
                """Emit one half of window wi / branch br's scatter matmuls;
                on the second half, copy the PSUM row-block to bf16 SBUF."""
                if wi is None:
                    return
                w0, wl = wins[wi]
                chunks = st["win_chunks"][wi]
                total = len(chunks)
                lo, hi = (0, total // 2) if half == 0 else (total // 2, total)
                if half == 0:
                    sc_state[(wi, br)] = (
                        pscat.tile([128, cfg.WIN], F32, tag="psc",
                                   name=f"psc{wi}_{br}"),
                        emit_onehot(wi, br),
                    )
                psc, s = sc_state[(wi, br)]
                xw = win_tiles[wi][br]
                scol = sum(c[1] for c in chunks[:lo])
                for idx in range(lo, hi):
                    off, wd, sta, sto = chunks[idx]
                    nc.tensor.matmul(
                        out=psc[:, off:off + wd],
                        lhsT=xw[:, idx * 128:(idx + 1) * 128],
                        rhs=s[:, scol:scol + wd],
                        start=sta, stop=sto)
                    scol += wd
                if half == 1:
                    ag = aggp.tile([128, cfg.WIN], BF16, tag=f"agg{br}")
                    nc.vector.tensor_copy(out=ag[:, :wl], in_=psc[:, :wl])
                    aggw_store[(wi, br)] = ag
                    del sc_state[(wi, br)]

            def emit_merge(wi, cur=None, tps=None):
                """Merge window wi: concat(h0,h1) @ Wm + bm -> out rows."""
                w0, wl = wins[wi]
                nb = -(-wl // 128)
                curw = hfin_store.pop(wi) if cur is None else cur
                for tp in (range(0, nb, 2) if tps is None else tps):
                    seg_n = min(2, nb - tp)
                    po = pmerge.tile([128, cfg.WIN], F32, tag="po")
                    for s in range(seg_n):
                        r0 = (tp + s) * 128
                        rl = min(128, wl - r0)
                        si = r0 // 256
                        rr = r0 - si * 256
                        for ic in range(4):
                            nc.tensor.matmul(
                                out=po[:rl, s * 256:s * 256 + cfg.OUT_CH],
                                lhsT=curw[(ic // 2, si)][:, (ic % 2) * 256 + rr:
                                                         (ic % 2) * 256 + rr + rl],
                                rhs=wm_sb[:, ic * cfg.OUT_CH:(ic + 1) * cfg.OUT_CH],
                                start=(ic == 0), stop=(ic == 3))
                    o_sb = outp.tile([128, 2 * cfg.OUT_CH], BF16, tag="o")
                    width = seg_n * 256
                    nc.vector.tensor_tensor(
                        out=o_sb[:, :width], in0=po[:, :width],
                        in1=bmt_sb[:, :width], op=mybir.AluOpType.add)
                    for s in range(seg_n):
                        r0 = (tp + s) * 128
                        rl = min(128, wl - r0)
                        # round-robin output writes across three queues so the
                        # tail flush isn't serialized on one engine
                        eng = (nc.gpsimd, nc.scalar)[out_rr[0] % 2]
                        out_rr[0] += 1
                        eng.dma_start(
                            out=out_d[w0 + r0:w0 + r0 + rl, :],
                            in_=o_sb[:rl, s * 256:s * 256 + 256])

            # prologue: scatter the first window fully
            for br in (0, 1):
                emit_sc_half(worder[0], br, 0)
                emit_sc_half(worder[0], br, 1)

            for pos, wi in enumerate(worder):
                w0, wl = wins[wi]
                last_pos = pos == len(worder) - 1
                nxt = worder[pos + 1] if pos + 1 < len(worder) else None
                if pos + 2 < len(worder):
                    win_tiles[worder[pos + 2]] = load_win(worder[pos + 2])
                # previous window's merge first: it is dependency-free, so it
                # fills the PE queue while this window's agg is still landing
                if pos > 0:
                    emit_merge(worder[pos - 1])
                aggw = [aggw_store.pop((wi, 0)), aggw_store.pop((wi, 1))]
                xt_w = win_tiles[wi][2]

                subs = [(0, min(256, wl))]
                if wl > 256:
                    subs.append((256, wl - 256))
                chains = [(br, si) for br in (0, 1) for si in range(len(subs))]
                cur = {}
                for br, si in chains:
                    s0, sl = subs[si]
                    ph = pdense.tile([128, cfg.WIN], F32, tag="pd",
                                     name=f"ph{br}{si}")
                    for ocb in range(2):
                        nc.tensor.matmul(
                            out=ph[:, ocb * 256:ocb * 256 + sl],
                            lhsT=w01_sb[:, br * 256 + ocb * 128:
                                        br * 256 + ocb * 128 + 128],
                            rhs=aggw[br][:, s0:s0 + sl],
                            start=True, stop=True)
                    h = hwin.tile([128, cfg.WIN], BF16, tag=f"h{br}{si}",
                                  name=f"h{br}{si}")
                    nc.vector.tensor_tensor(
                        out=h[:], in0=ph[:],
                        in1=xt_w[:, si * 512:si * 512 + 512],
                        op=mybir.AluOpType.add)
                    cur[(br, si)] = h
                emit_sc_half(nxt, 0, 0)
                for l in range(cfg.N_MLP):
                    last = l == cfg.N_MLP - 1
                    # on the very last window, finish chains si-major and
                    # interleave its merge groups so the tail isn't serial
                    lchains = (sorted(chains, key=lambda c: (c[1], c[0]))
                               if (last and last_pos) else chains)
                    for ci, (br, si) in enumerate(lchains):
                        s0, sl = subs[si]
                        pm = pdense.tile([128, cfg.WIN], F32, tag="pd",
                                         name=f"pm{l}{br}{si}")
                        for ocb in range(2):
                            for icb in range(2):
                                nc.tensor.matmul(
                                    out=pm[:, ocb * 256:ocb * 256 + sl],
                                    lhsT=mlpw_sb[br][:, (l * 4 + icb * 2 + ocb) * 128:
                                                     (l * 4 + icb * 2 + ocb) * 128 + 128],
                                    rhs=cur[(br, si)][:, icb * 256:icb * 256 + sl],
                                    start=(icb == 0), stop=(icb == 1))
                        pool = hfin if last else hwin
                        hn = pool.tile([128, cfg.WIN], BF16,
                                       tag=(f"hf{br}{si}" if last
                                            else f"h{br}{si}"),
                                       name=f"hn{l}{br}{si}")
                        if zero_bias:
                            nc.scalar.activation(out=hn[:], in_=pm[:],
                                                 func=RELU, bias=0.0)
                        else:
                            for ocb in range(2):
                                nc.scalar.activation(
                                    out=hn[:, ocb * 256:(ocb + 1) * 256],
                                    in_=pm[:, ocb * 256:(ocb + 1) * 256],
                                    func=RELU,
                                    bias=b_sb[br][:, l * 2 + ocb:l * 2 + ocb + 1])
                        cur[(br, si)] = hn
                        if last and last_pos and ci % 2 == 1:
                            emit_merge(wi, cur=cur, tps=[2 * si])
                    if l == 0:
                        emit_sc_half(nxt, 0, 1)
                        emit_sc_half(nxt, 1, 0)
                    elif l == 1:
                        emit_sc_half(nxt, 1, 1)
                if not last_pos:
                    hfin_store[wi] = cur

    nc.compile()
    return nc


# -------------------------------------------------------------------- runner

_CACHE = {}


def kernel(**inputs) -> np.ndarray:
    _install_profile_hook()
    cfg = CFG
    in_maps, st, zero_bias, perm = prep_inputs(cfg, inputs)
    key = ("v13", tuple(int(v) for v in st["k"]), zero_bias)
    if key not in _CACHE:
        _CACHE[key] = build(cfg, st, zero_bias)
    nc = _CACHE[key]
    trace = bool(int(__import__("os").environ.get("KERNEL_TRACE", "0")))
    r = run_bass_kernel_spmd(nc, in_maps, core_ids=list(range(cfg.NC)), trace=trace)
    kernel.last_result = r
    res = np.concatenate([r.results[c]["out"] for c in range(cfg.NC)], axis=0)
    out = np.empty((cfg.N_T, cfg.OUT_CH), np.float32)
    out[perm] = res
    return out


kernel.last_result = None


# revision 26
# speedup vs baseline: 1.0135x; 1.0135x over previous
"""Trainium2 Bass kernel for nn_CXINGeneral_1425929142863 (GNN message passing).

Math (per branch b, epsilon=0):
    agg_b = A_b @ x_src_b               (gather + segment-sum, IN_CH=128 space)
    h_b   = relu-MLP_b( agg_b @ W_b + x_target )      (3 layers)
    out   = concat(h0, h1) @ Wm + bm

Key rewrite: A @ (x_src @ W) == (A @ x_src) @ W — aggregate in IN_CH space,
making every dense matmul local to the target shard. Target rows sharded 8
ways; all weights replicated; no collectives.

v13 layout (vs v12's DMA'd one-hot stream):
  - The scatter one-hot S is built ON DEVICE: per edge we ship only a
    bf16 block-local row index d (2 B/edge); S = is_equal(iota, d) via one
    broadcast DVE op per window/branch. Saves ~28 MB/core of HBM reads.
  - Narrow scatter slots (28 rows instead of 128). Matmul cost on TensorE is
    out-width cycles, so scatter PE time drops 4x. To avoid ceil-padding
    blowup at small widths, target rows are PERMUTED host-side: a greedy
    min-max bin packer balances per-slot edge counts across BOTH branches so
    every 28-row slot needs exactly ceil(<=256/128)=2 chunks of 128 edges.
    The host inverse-permutes the output rows at the end.
  - Output written bf16 (host upcasts; ~0.1% rms, well inside tolerance).

Per window (512 rows = 18 slots x 28 + 1 slot x 8; last window 3x28+22):
  scatter chunks accumulate X_chunk.T @ S_chunk into PSUM [128ch, rows];
  dense (transposed activations [ch, rows]): head matmul + x_target add,
  3x (matmul + relu on ACT); merge: 4 accumulating matmuls + bias add.
Branch 0/1 work is interleaved so TensorE never waits on ACT/DVE.
"""

import heapq
import sys
import types

import numpy as np
import ml_dtypes

import concourse.bass as bass
import concourse.mybir as mybir
import concourse.tile as tile
from concourse import bacc
import concourse.bass_utils as bass_utils
from concourse.bass_utils import run_bass_kernel_spmd

F32 = mybir.dt.float32
BF16 = mybir.dt.bfloat16
I32 = mybir.dt.int32
BF16_NP = ml_dtypes.bfloat16


def _install_profile_hook():
    """This container's antenv lacks axon_hooks; reconstruct so trace=True works."""
    try:
        import antenv.axon_hooks  # noqa: F401
        return
    except ImportError:
        pass
    try:
        from trn_agent_boot.trn_boot import _ntff_profile_via_ctypes
    except ImportError:
        return
    mod = types.ModuleType("antenv.axon_hooks")
    hook = _ntff_profile_via_ctypes("/opt/axon/libaxon_pjrt.so")
    mod.get_axon_ntff_profile_hook = lambda: hook
    sys.modules["antenv.axon_hooks"] = mod
    bass_utils.upload_artifacts = lambda tmpdir: f"local:{tmpdir}"


class Cfg:
    def __init__(self):
        self.N_T = 50000
        self.N_S = 100000
        self.E = 400000
        self.NC = 8
        self.IN_CH = 128
        self.OUT_CH = 256
        self.N_MLP = 3
        self.NT_LOC = self.N_T // self.NC      # 6250
        self.WIN = 512
        self.SW = 28                            # big slot width
        # per-core grid: 12 windows of (18x28 + 1x8), 1 window of (3x28 + 1x22)
        self.grid = []
        for _ in range(12):
            self.grid.append([(28, i * 28) for i in range(18)] + [(8, 504)])
        self.grid.append([(28, i * 28) for i in range(3)] + [(22, 84)])
        self.n_wins = len(self.grid)            # 13
        self.slots_per_win = [len(w) for w in self.grid]
        self.n_gslots = sum(self.slots_per_win)  # 232
        # gslot id = running index in (window, slot) order
        self.gslot_base = np.cumsum([0] + self.slots_per_win).astype(np.int64)


CFG = Cfg()


# ----------------------------------------------------------------- host prep

def _balance(cfg, deg0, deg1):
    """Permute target rows so each (core, window, slot) bin has balanced edge
    counts in BOTH branches. Returns perm (new->old) and per-bin row lists
    keyed by (core, win, slot)."""
    total = deg0 + deg1
    order_asc = np.argsort(total, kind="stable")

    n_small8 = cfg.NC * 12          # 96 bins of 8 rows
    n_tail22 = cfg.NC               # 8 bins of 22 rows
    n_small_rows = n_small8 * 8 + n_tail22 * 22   # 944
    small_rows = order_asc[:n_small_rows]
    big_rows = order_asc[n_small_rows:][::-1]     # descending degree

    # --- greedy min-max packing into 28-row bins
    n_big = cfg.NC * (12 * 18 + 3)  # 1752
    heap = [(0, 0, 0, 0, b) for b in range(n_big)]
    heapq.heapify(heap)
    assign = [[] for _ in range(n_big)]
    loads = [(0, 0)] * n_big
    d0l = deg0[big_rows]
    d1l = deg1[big_rows]
    for i in range(len(big_rows)):
        r = big_rows[i]
        _, l0, l1, n, b = heapq.heappop(heap)
        assign[b].append(r)
        l0 += int(d0l[i])
        l1 += int(d1l[i])
        n += 1
        loads[b] = (l0, l1)
        if n < 28:
            heapq.heappush(heap, (max(l0, l1), l0, l1, n, b))

    # heavy bins grouped 8-at-a-time onto the same (win, slot) across cores
    bin_order = sorted(range(n_big), key=lambda b: -max(*loads[b]))
    big_positions = [(w, s) for w in range(12) for s in range(18)] + \
                    [(12, s) for s in range(3)]
    bins = {}
    for g, (w, s) in enumerate(big_positions):
        for c in range(cfg.NC):
            bins[(c, w, s)] = assign[bin_order[g * cfg.NC + c]]

    # small bins: lowest-degree rows, sequential fill
    p = 0
    for i in range(n_small8):
        c, w = i % cfg.NC, i // cfg.NC
        bins[(c, w, 18)] = list(small_rows[p:p + 8])
        p += 8
    for c in range(cfg.NC):
        bins[(c, 12, 3)] = list(small_rows[p:p + 22])
        p += 22

    perm = np.empty(cfg.N_T, np.int64)
    for c in range(cfg.NC):
        base = c * cfg.NT_LOC
        for w in range(cfg.n_wins):
            for s, (wd, roff) in enumerate(cfg.grid[w]):
                rows = bins[(c, w, s)]
                assert len(rows) == wd
                perm[base + w * cfg.WIN + roff:
                     base + w * cfg.WIN + roff + wd] = rows
    return perm


def _edge_fields(cfg, rows_new):
    """Map permuted row index -> (core, win, gslot, local_d)."""
    core = rows_new // cfg.NT_LOC
    lrow = rows_new % cfg.NT_LOC
    win = lrow // cfg.WIN
    wrow = lrow % cfg.WIN
    n28 = np.where(win < 12, 18, 3)
    s_idx = np.minimum(wrow // cfg.SW, n28)
    roff = np.where(s_idx < n28, s_idx * cfg.SW, n28 * cfg.SW)
    local_d = wrow - roff
    gslot = cfg.gslot_base[win] + s_idx
    return core, win, gslot, local_d


def _structure(cfg, k):
    """Derive chunk-stream layout from per-gslot chunk counts k (len 232).

    Returns dict with per-window chunk descriptor lists and bases."""
    k = np.asarray(k, np.int64)
    st = {"k": k}
    chunk_base = np.zeros(cfg.n_gslots, np.int64)
    np.cumsum(k[:-1], out=chunk_base[1:])
    st["chunk_base"] = chunk_base
    st["K_total"] = int(k.sum())
    win_chunks = []     # per window: list of (psc_off, width, start, stop)
    win_base = []       # first chunk col of window
    n28_l, ksm_l, smw_l, swidth_l = [], [], [], []
    for w in range(cfg.n_wins):
        g0 = cfg.gslot_base[w]
        chunks = []
        scol = 0
        n28 = 0
        for s, (wd, roff) in enumerate(cfg.grid[w]):
            kk = int(k[g0 + s])
            for i in range(kk):
                chunks.append((roff, wd, i == 0, i == kk - 1))
            if wd == cfg.SW:
                n28 += kk
            else:
                ksm_l.append(kk)
                smw_l.append(wd)
            scol += kk * wd
        win_chunks.append(chunks)
        win_base.append(int(chunk_base[g0]))
        n28_l.append(n28)
        swidth_l.append(scol)
    st["win_chunks"] = win_chunks
    st["win_base"] = win_base
    st["n28"] = n28_l
    st["ksm"] = ksm_l          # per window small-slot chunk count
    st["smw"] = smw_l          # per window small-slot width (8 or 22)
    st["s_width"] = swidth_l   # one-hot tile cols per window
    st["Kwin_max"] = max(len(c) for c in win_chunks)
    st["swidth_max"] = max(swidth_l)
    st["n28_max"] = max(n28_l)
    st["k8_max"] = max(ksm_l[w] for w in range(12))
    st["k22_max"] = ksm_l[12]
    return st


def prep_inputs(cfg, inputs):
    deg0 = np.bincount(np.asarray(inputs["rows0"], np.int64), minlength=cfg.N_T)
    deg1 = np.bincount(np.asarray(inputs["rows1"], np.int64), minlength=cfg.N_T)
    perm = _balance(cfg, deg0, deg1)
    pos = np.empty(cfg.N_T, np.int64)
    pos[perm] = np.arange(cfg.N_T)

    # per-branch edge fields + per-gslot counts (max over cores+branches)
    br_fields = []
    counts = np.zeros((2, cfg.NC, cfg.n_gslots), np.int64)
    for b in (0, 1):
        rows_new = pos[np.asarray(inputs[f"rows{b}"], np.int64)]
        core, win, gslot, local_d = _edge_fields(cfg, rows_new)
        np.add.at(counts[b], (core, gslot), 1)
        br_fields.append((core, gslot, local_d))
    k = np.maximum(1, -(-counts.max(axis=(0, 1)) // 128))
    st = _structure(cfg, k)
    K = st["K_total"]
    chunk_base = st["chunk_base"]

    # pack x / d streams
    xd = {}
    for b in (0, 1):
        core, gslot, local_d = br_fields[b]
        cols = np.asarray(inputs[f"cols{b}"], np.int64)
        vals = np.asarray(inputs[f"vals{b}"], np.float32)
        xsrc = np.asarray(inputs[f"x_src{b}"], np.float32)
        key = core * cfg.n_gslots + gslot
        order = np.argsort(key, kind="stable")
        key_s = key[order]
        grp_counts = np.bincount(key_s, minlength=cfg.NC * cfg.n_gslots)
        starts = np.zeros(cfg.NC * cfg.n_gslots, np.int64)
        np.cumsum(grp_counts[:-1], out=starts[1:])
        rank = np.arange(len(key)) - starts[key_s]
        core_s = core[order]
        chunkcol = chunk_base[gslot[order]] + rank // 128
        lane = rank % 128
        x_arr = np.zeros((cfg.NC, 128, K, 128), BF16_NP)
        d_arr = np.zeros((cfg.NC, 128, K), BF16_NP)
        x_arr[core_s, lane, chunkcol] = \
            (vals[order][:, None] * xsrc[cols[order]]).astype(BF16_NP)
        d_arr[core_s, lane, chunkcol] = local_d[order].astype(BF16_NP)
        xd[f"x{b}"] = x_arr
        xd[f"d{b}"] = d_arr

    zero_bias = (not np.any(np.asarray(inputs["mlp_b0"]))
                 and not np.any(np.asarray(inputs["mlp_b1"])))

    # ---- weights (same layouts as v12)
    x_target = np.asarray(inputs["x_target"], np.float32)
    W0 = np.asarray(inputs["W0"], np.float32)
    W1 = np.asarray(inputs["W1"], np.float32)
    w01 = np.ascontiguousarray(np.concatenate([W0, W1], axis=1)).astype(BF16_NP)

    mlpw = []
    for b in (0, 1):
        mw = np.asarray(inputs[f"mlp_W{b}"], np.float32)
        blocks = []
        for l in range(cfg.N_MLP):
            for icb in range(2):
                for ocb in range(2):
                    blocks.append(mw[l, icb * 128:(icb + 1) * 128,
                                     ocb * 128:(ocb + 1) * 128])
        mlpw.append(np.ascontiguousarray(
            np.concatenate(blocks, axis=1)).astype(BF16_NP))

    mlpb = []
    for b in (0, 1):
        mb_ = np.asarray(inputs[f"mlp_b{b}"], np.float32)
        cols_ = []
        for l in range(cfg.N_MLP):
            for ocb in range(2):
                cols_.append(mb_[l, ocb * 128:(ocb + 1) * 128][:, None])
        mlpb.append(np.ascontiguousarray(np.concatenate(cols_, axis=1)))

    Wm = np.asarray(inputs["Wm"], np.float32)
    wm = np.ascontiguousarray(
        np.concatenate([Wm[i * 128:(i + 1) * 128, :] for i in range(4)], axis=1)
    ).astype(BF16_NP)
    bmt = np.ascontiguousarray(
        np.tile(np.asarray(inputs["bm"], np.float32), (128, 2)))

    in_maps = []
    for c in range(cfg.NC):
        xt_loc = x_target[perm[c * cfg.NT_LOC:(c + 1) * cfg.NT_LOC]]
        xt = np.zeros((128, cfg.n_wins * 2 * cfg.WIN), BF16_NP)
        for wi in range(cfg.n_wins):
            w0 = wi * cfg.WIN
            wl = min(cfg.WIN, cfg.NT_LOC - w0)
            for sub in range(2):
                s0 = sub * 256
                if s0 >= wl:
                    break
                sl = min(256, wl - s0)
                for ocb in range(2):
                    base = wi * 2 * cfg.WIN + sub * 512 + ocb * 256
                    xt[:, base:base + sl] = \
                        xt_loc[w0 + s0:w0 + s0 + sl,
                               ocb * 128:(ocb + 1) * 128].T
        m = {
            "xt": xt,
            "w01": w01, "mlpw0": mlpw[0], "mlpw1": mlpw[1],
            "b0": mlpb[0], "b1": mlpb[1],
            "wm": wm, "bmt": bmt,
        }
        for b in (0, 1):
            m[f"x{b}"] = np.ascontiguousarray(xd[f"x{b}"][c]).reshape(128, K * 128)
            m[f"d{b}"] = np.ascontiguousarray(xd[f"d{b}"][c])
        in_maps.append(m)
    return in_maps, st, zero_bias, perm


# ------------------------------------------------------------------- builder

def build(cfg, st, zero_bias):
    nc = bacc.Bacc("TRN2", target_bir_lowering=False, debug=False)

    K = st["K_total"]
    n_wins = cfg.n_wins
    x_d = [nc.declare_dram_parameter(f"x{b}", [128, K * 128], BF16, isOutput=False)
           for b in (0, 1)]
    d_d = [nc.declare_dram_parameter(f"d{b}", [128, K], BF16, isOutput=False)
           for b in (0, 1)]
    xt_d = nc.declare_dram_parameter("xt", [128, n_wins * 2 * cfg.WIN], BF16,
                                     isOutput=False)
    w01_d = nc.declare_dram_parameter("w01", [128, 512], BF16, isOutput=False)
    mlpw_d = [nc.declare_dram_parameter(f"mlpw{b}", [128, cfg.N_MLP * 4 * 128], BF16,
                                        isOutput=False) for b in (0, 1)]
    b_d = [nc.declare_dram_parameter(f"b{b}", [128, cfg.N_MLP * 2], F32, isOutput=False)
           for b in (0, 1)]
    wm_d = nc.declare_dram_parameter("wm", [128, 4 * cfg.OUT_CH], BF16, isOutput=False)
    bmt_d = nc.declare_dram_parameter("bmt", [128, 2 * cfg.OUT_CH], F32, isOutput=False)
    out_d = nc.declare_dram_parameter("out", [cfg.NT_LOC, cfg.OUT_CH], BF16,
                                      isOutput=True)

    wins = []
    w0 = 0
    while w0 < cfg.NT_LOC:
        wins.append((w0, min(cfg.WIN, cfg.NT_LOC - w0)))
        w0 += cfg.WIN

    RELU = mybir.ActivationFunctionType.Relu
    EQ = mybir.AluOpType.is_equal

    with tile.TileContext(nc) as tc:
        with (
            tc.tile_pool(name="wpool", bufs=1) as wpool,
            tc.tile_pool(name="xwin", bufs=3) as xpool,
            tc.tile_pool(name="xtwin", bufs=3) as xtpool,
            tc.tile_pool(name="swin", bufs=2) as spool,
            tc.tile_pool(name="aggp", bufs=2) as aggp,
            tc.tile_pool(name="hwin", bufs=2) as hwin,
            tc.tile_pool(name="hfin", bufs=2) as hfin,
            tc.tile_pool(name="outp", bufs=2) as outp,
            tc.tile_pool(name="pscat", bufs=2, space="PSUM") as pscat,
            tc.tile_pool(name="pdense", bufs=4, space="PSUM") as pdense,
            tc.tile_pool(name="pmerge", bufs=2, space="PSUM") as pmerge,
        ):
            # --- per-edge index streams first: the prologue scatter needs
            # only d + the first window's x, so keep big weight DMAs behind.
            d_sb = []
            for b in (0, 1):
                td = wpool.tile([128, K], BF16, tag=f"d{b}")
                nc.sync.dma_start(out=td[:], in_=d_d[b][:])
                d_sb.append(td)

            # --- iota tiles for on-device one-hot build
            io28 = wpool.tile([128, st["n28_max"] * 28], BF16, tag="io28")
            nc.gpsimd.iota(out=io28[:], pattern=[[0, st["n28_max"]], [1, 28]],
                           base=0, channel_multiplier=0,
                           allow_small_or_imprecise_dtypes=True)
            io8 = wpool.tile([128, st["k8_max"] * 8], BF16, tag="io8")
            nc.gpsimd.iota(out=io8[:], pattern=[[0, st["k8_max"]], [1, 8]],
                           base=0, channel_multiplier=0,
                           allow_small_or_imprecise_dtypes=True)
            io22 = wpool.tile([128, st["k22_max"] * 22], BF16, tag="io22")
            nc.gpsimd.iota(out=io22[:], pattern=[[0, st["k22_max"]], [1, 22]],
                           base=0, channel_multiplier=0,
                           allow_small_or_imprecise_dtypes=True)

            def load_win(wi):
                """Prefetch window wi's x chunk streams (both branches, split
                into two half-DMAs so scatter can start on the first half)
                and its x_target slice."""
                wb = st["win_base"][wi]
                kw = len(st["win_chunks"][wi])
                half = kw // 2
                tiles = []
                for br in (0, 1):
                    xw = xpool.tile([128, st["Kwin_max"] * 128], BF16, tag=f"xw{br}")
                    xh = [xw[:, :half * 128], xw[:, half * 128:kw * 128]]
                    nc.sync.dma_start(
                        out=xh[0],
                        in_=x_d[br][:, wb * 128:(wb + half) * 128])
                    nc.sync.dma_start(
                        out=xh[1],
                        in_=x_d[br][:, (wb + half) * 128:(wb + kw) * 128])
                    tiles.append(xw)
                xt_w = xtpool.tile([128, 2 * cfg.WIN], BF16, tag="xt")
                nc.sync.dma_start(
                    out=xt_w[:],
                    in_=xt_d[:, wi * 2 * cfg.WIN:(wi + 1) * 2 * cfg.WIN])
                tiles.append(xt_w)
                return tiles

            def emit_onehot(wi, br):
                """Build window wi's one-hot scatter tile for branch br."""
                wb = st["win_base"][wi]
                n28 = st["n28"][wi]
                ksm = st["ksm"][wi]
                smw = st["smw"][wi]
                s = spool.tile([128, st["swidth_max"]], BF16, tag=f"s{br}")
                nc.vector.tensor_tensor(
                    out=s[:, :n28 * 28].rearrange("p (n w) -> p n w", w=28),
                    in0=io28[:, :n28 * 28].rearrange("p (n w) -> p n w", w=28),
                    in1=d_sb[br][:, wb:wb + n28].unsqueeze(2)
                        .broadcast_to([128, n28, 28]),
                    op=EQ)
                off = n28 * 28
                iot = io8 if smw == 8 else io22
                nc.vector.tensor_tensor(
                    out=s[:, off:off + ksm * smw]
                        .rearrange("p (n w) -> p n w", w=smw),
                    in0=iot[:, :ksm * smw].rearrange("p (n w) -> p n w", w=smw),
                    in1=d_sb[br][:, wb + n28:wb + n28 + ksm].unsqueeze(2)
                        .broadcast_to([128, ksm, smw]),
                    op=EQ)
                return s

            # process the short (106-row) window FIRST: its x stream is tiny
            # (fast pipeline fill while weights stream in) and the final
            # window then has 4 full dependency chains (no 2-chain tail).
            worder = [len(wins) - 1] + list(range(len(wins) - 1))
            win_tiles = {worder[0]: load_win(worder[0])}

            # weights ride the scalar queue so they never block the x stream
            w01_sb = wpool.tile([128, 512], BF16, tag="w01")
            nc.scalar.dma_start(out=w01_sb[:], in_=w01_d[:])
            win_tiles[worder[1]] = load_win(worder[1])
            mlpw_sb, b_sb = [], []
            for b in (0, 1):
                t = wpool.tile([128, cfg.N_MLP * 4 * 128], BF16, tag=f"mlpw{b}")
                for l in range(cfg.N_MLP):
                    nc.scalar.dma_start(
                        out=t[:, l * 512:(l + 1) * 512],
                        in_=mlpw_d[b][:, l * 512:(l + 1) * 512])
                mlpw_sb.append(t)
                if not zero_bias:
                    tb = wpool.tile([128, cfg.N_MLP * 2], F32, tag=f"bias{b}")
                    nc.scalar.dma_start(out=tb[:], in_=b_d[b][:])
                    b_sb.append(tb)
            wm_sb = wpool.tile([128, 4 * cfg.OUT_CH], BF16, tag="wm")
            nc.scalar.dma_start(out=wm_sb[:], in_=wm_d[:])
            bmt_sb = wpool.tile([128, 2 * cfg.OUT_CH], F32, tag="bmt")
            nc.scalar.dma_start(out=bmt_sb[:], in_=bmt_d[:])

            hfin_store = {}
            sc_state = {}
            aggw_store = {}
            out_rr = [0]

            def emit_sc_half(wi, br, half):
                """Emit one half of window wi / branch br's scatter matmuls;
                on the second half, copy the PSUM row-block to bf16 SBUF."""
                if wi is None:
                    return
                w0, wl = wins[wi]
                chunks = st["win_chunks"][wi]
                total = len(chunks)
                lo, hi = (0, total // 2) if half == 0 else (total // 2, total)
                if half == 0:
                    sc_state[(wi, br)] = (
                        pscat.tile([128, cfg.WIN], F32, tag="psc",
                                   name=f"psc{wi}_{br}"),
                        emit_onehot(wi, br),
                    )
                psc, s = sc_state[(wi, br)]
                xw = win_tiles[wi][br]
                scol = sum(c[1] for c in chunks[:lo])
                for idx in range(lo, hi):
                    off, wd, sta, sto = chunks[idx]
                    nc.tensor.matmul(
                        out=psc[:, off:off + wd],
                        lhsT=xw[:, idx * 128:(idx + 1) * 128],
                        rhs=s[:, scol:scol + wd],
                        start=sta, stop=sto)
                    scol += wd
                if half == 1:
                    ag = aggp.tile([128, cfg.WIN], BF16, tag=f"agg{br}")
                    nc.vector.tensor_copy(out=ag[:, :wl], in_=psc[:, :wl])
                    aggw_store[(wi, br)] = ag
                    del sc_state[(wi, br)]

            def emit_merge(wi, cur=None, tps=None):
                """Merge window wi: concat(h0,h1) @ Wm + bm -> out rows."""
                w0, wl = wins[wi]
                nb = -(-wl // 128)
                curw = hfin_store.pop(wi) if cur is None else cur
                for tp in (range(0, nb, 2) if tps is None else tps):
                    seg_n = min(2, nb - tp)
                    po = pmerge.tile([128, cfg.WIN], F32, tag="po")
                    for s in range(seg_n):
                        r0 = (tp + s) * 128
                        rl = min(128, wl - r0)
                        si = r0 // 256
                        rr = r0 - si * 256
                        for ic in range(4):
                            nc.tensor.matmul(
                                out=po[:rl, s * 256:s * 256 + cfg.OUT_CH],
                                lhsT=curw[(ic // 2, si)][:, (ic % 2) * 256 + rr:
                                                         (ic % 2) * 256 + rr + rl],
                                rhs=wm_sb[:, ic * cfg.OUT_CH:(ic + 1) * cfg.OUT_CH],
                                start=(ic == 0), stop=(ic == 3))
                    o_sb = outp.tile([128, 2 * cfg.OUT_CH], BF16, tag="o")
                    width = seg_n * 256
                    nc.vector.tensor_tensor(
                        out=o_sb[:, :width], in0=po[:, :width],
                        in1=bmt_sb[:, :width], op=mybir.AluOpType.add)
                    for s in range(seg_n):
                        r0 = (tp + s) * 128
                        rl = min(128, wl - r0)
                        # round-robin output writes across three queues so the
                        # tail flush isn't serialized on one engine
                        eng = nc.gpsimd
                        out_rr[0] += 1
                        eng.dma_start(
                            out=out_d[w0 + r0:w0 + r0 + rl, :],
                            in_=o_sb[:rl, s * 256:s * 256 + 256])

            # prologue: scatter the first window fully
            for br in (0, 1):
                emit_sc_half(worder[0], br, 0)
                emit_sc_half(worder[0], br, 1)

            for pos, wi in enumerate(worder):
                w0, wl = wins[wi]
                last_pos = pos == len(worder) - 1
                nxt = worder[pos + 1] if pos + 1 < len(worder) else None
                if pos + 2 < len(worder):
                    win_tiles[worder[pos + 2]] = load_win(worder[pos + 2])
                aggw = [aggw_store.pop((wi, 0)), aggw_store.pop((wi, 1))]
                xt_w = win_tiles[wi][2]

                subs = [(0, min(256, wl))]
                if wl > 256:
                    subs.append((256, wl - 256))
                chains = [(br, si) for br in (0, 1) for si in range(len(subs))]
                cur = {}
                for br, si in chains:
                    s0, sl = subs[si]
                    ph = pdense.tile([128, cfg.WIN], F32, tag="pd",
                                     name=f"ph{br}{si}")
                    for ocb in range(2):
                        nc.tensor.matmul(
                            out=ph[:, ocb * 256:ocb * 256 + sl],
                            lhsT=w01_sb[:, br * 256 + ocb * 128:
                                        br * 256 + ocb * 128 + 128],
                            rhs=aggw[br][:, s0:s0 + sl],
                            start=True, stop=True)
                    h = hwin.tile([128, cfg.WIN], BF16, tag=f"h{br}{si}",
                                  name=f"h{br}{si}")
                    nc.vector.tensor_tensor(
                        out=h[:], in0=ph[:],
                        in1=xt_w[:, si * 512:si * 512 + 512],
                        op=mybir.AluOpType.add)
                    cur[(br, si)] = h
                emit_sc_half(nxt, 0, 0)
                for l in range(cfg.N_MLP):
                    last = l == cfg.N_MLP - 1
                    # on the very last window, finish chains si-major and
                    # interleave its merge groups so the tail isn't serial
                    lchains = (sorted(chains, key=lambda c: (c[1], c[0]))
                               if (last and last_pos) else chains)
                    for ci, (br, si) in enumerate(lchains):
                        s0, sl = subs[si]
                        pm = pdense.tile([128, cfg.WIN], F32, tag="pd",
                                         name=f"pm{l}{br}{si}")
                        for ocb in range(2):
                            for icb in range(2):
                                nc.tensor.matmul(
                                    out=pm[:, ocb * 256:ocb * 256 + sl],
                                    lhsT=mlpw_sb[br][:, (l * 4 + icb * 2 + ocb) * 128:
                                                     (l * 4 + icb * 2 + ocb) * 128 + 128],
                                    rhs=cur[(br, si)][:, icb * 256:icb * 256 + sl],
                                    start=(icb == 0), stop=(icb == 1))
                        pool = hfin if last else hwin
                        hn = pool.tile([128, cfg.WIN], BF16,
                                       tag=(f"hf{br}{si}" if last
                                            else f"h{br}{si}"),
                                       name=f"hn{l}{br}{si}")
                        if zero_bias:
                            nc.scalar.activation(out=hn[:], in_=pm[:],
                                                 func=RELU, bias=0.0)
                        else:
                            for ocb in range(2):
                                nc.scalar.activation(
                                    out=hn[:, ocb * 256:(ocb + 1) * 256],
                                    in_=pm[:, ocb * 256:(ocb + 1) * 256],
                                    func=RELU,
                                    bias=b_sb[br][:, l * 2 + ocb:l * 2 + ocb + 1])
                        cur[(br, si)] = hn
                        if last and last_pos and ci % 2 == 1:
                            emit_merge(wi, cur=cur, tps=[2 * si])
                    if l == 0:
                        emit_sc_half(nxt, 0, 1)
                        emit_sc_half(nxt, 1, 0)
                    elif l == 1:
                        emit_sc_half(nxt, 1, 1)
                        if pos > 0:
                            emit_merge(worder[pos - 1])
                if not last_pos:
                    hfin_store[wi] = cur

    nc.compile()
    return nc


# -------------------------------------------------------------------- runner

_CACHE = {}


def kernel(**inputs) -> np.ndarray:
    _install_profile_hook()
    cfg = CFG
    in_maps, st, zero_bias, perm = prep_inputs(cfg, inputs)
    key = ("v13", tuple(int(v) for v in st["k"]), zero_bias)
    if key not in _CACHE:
        _CACHE[key] = build(cfg, st, zero_bias)
    nc = _CACHE[key]
    trace = bool(int(__import__("os").environ.get("KERNEL_TRACE", "0")))
    r = run_bass_kernel_spmd(nc, in_maps, core_ids=list(range(cfg.NC)), trace=trace)
    kernel.last_result = r
    res = np.concatenate([r.results[c]["out"] for c in range(cfg.NC)], axis=0)
    out = np.empty((cfg.N_T, cfg.OUT_CH), np.float32)
    out[perm] = res
    return out


kernel.last_result = None


# revision 30
# speedup vs baseline: 1.0333x; 1.0196x over previous
"""Trainium2 Bass kernel for nn_CXINGeneral_1425929142863 (GNN message passing).

Math (per branch b, epsilon=0):
    agg_b = A_b @ x_src_b               (gather + segment-sum, IN_CH=128 space)
    h_b   = relu-MLP_b( agg_b @ W_b + x_target )      (3 layers)
    out   = concat(h0, h1) @ Wm + bm

Key rewrite: A @ (x_src @ W) == (A @ x_src) @ W — aggregate in IN_CH space,
making every dense matmul local to the target shard. Target rows sharded 8
ways; all weights replicated; no collectives.

v13 layout (vs v12's DMA'd one-hot stream):
  - The scatter one-hot S is built ON DEVICE: per edge we ship only a
    bf16 block-local row index d (2 B/edge); S = is_equal(iota, d) via one
    broadcast DVE op per window/branch. Saves ~28 MB/core of HBM reads.
  - Narrow scatter slots (28 rows instead of 128). Matmul cost on TensorE is
    out-width cycles, so scatter PE time drops 4x. To avoid ceil-padding
    blowup at small widths, target rows are PERMUTED host-side: a greedy
    min-max bin packer balances per-slot edge counts across BOTH branches so
    every 28-row slot needs exactly ceil(<=256/128)=2 chunks of 128 edges.
    The host inverse-permutes the output rows at the end.
  - Output written bf16 (host upcasts; ~0.1% rms, well inside tolerance).

Per window (512 rows = 18 slots x 28 + 1 slot x 8; last window 3x28+22):
  scatter chunks accumulate X_chunk.T @ S_chunk into PSUM [128ch, rows];
  dense (transposed activations [ch, rows]): head matmul + x_target add,
  3x (matmul + relu on ACT); merge: 4 accumulating matmuls + bias add.
Branch 0/1 work is interleaved so TensorE never waits on ACT/DVE.
"""

import heapq
import sys
import types

import numpy as np
import ml_dtypes

import concourse.bass as bass
import concourse.mybir as mybir
import concourse.tile as tile
from concourse import bacc
import concourse.bass_utils as bass_utils
from concourse.bass_utils import run_bass_kernel_spmd

F32 = mybir.dt.float32
BF16 = mybir.dt.bfloat16
I32 = mybir.dt.int32
BF16_NP = ml_dtypes.bfloat16


def _install_profile_hook():
    """This container's antenv lacks axon_hooks; reconstruct so trace=True works."""
    try:
        import antenv.axon_hooks  # noqa: F401
        return
    except ImportError:
        pass
    try:
        from trn_agent_boot.trn_boot import _ntff_profile_via_ctypes
    except ImportError:
        return
    mod = types.ModuleType("antenv.axon_hooks")
    hook = _ntff_profile_via_ctypes("/opt/axon/libaxon_pjrt.so")
    mod.get_axon_ntff_profile_hook = lambda: hook
    sys.modules["antenv.axon_hooks"] = mod
    bass_utils.upload_artifacts = lambda tmpdir: f"local:{tmpdir}"


class Cfg:
    def __init__(self):
        self.N_T = 50000
        self.N_S = 100000
        self.E = 400000
        self.NC = 8
        self.IN_CH = 128
        self.OUT_CH = 256
        self.N_MLP = 3
        self.NT_LOC = self.N_T // self.NC      # 6250
        self.WIN = 512
        self.SW = 28                            # big slot width
        # per-core grid: 12 windows of (18x28 + 1x8), 1 window of (3x28 + 1x22)
        self.grid = []
        for _ in range(12):
            self.grid.append([(28, i * 28) for i in range(18)] + [(8, 504)])
        self.grid.append([(28, i * 28) for i in range(3)] + [(22, 84)])
        self.n_wins = len(self.grid)            # 13
        self.slots_per_win = [len(w) for w in self.grid]
        self.n_gslots = sum(self.slots_per_win)  # 232
        # gslot id = running index in (window, slot) order
        self.gslot_base = np.cumsum([0] + self.slots_per_win).astype(np.int64)


CFG = Cfg()


# ----------------------------------------------------------------- host prep

def _balance(cfg, deg0, deg1):
    """Permute target rows so each (core, window, slot) bin has balanced edge
    counts in BOTH branches. Returns perm (new->old) and per-bin row lists
    keyed by (core, win, slot)."""
    total = deg0 + deg1
    order_asc = np.argsort(total, kind="stable")

    n_small8 = cfg.NC * 12          # 96 bins of 8 rows
    n_tail22 = cfg.NC               # 8 bins of 22 rows
    n_small_rows = n_small8 * 8 + n_tail22 * 22   # 944
    small_rows = order_asc[:n_small_rows]
    big_rows = order_asc[n_small_rows:][::-1]     # descending degree

    # --- greedy min-max packing into 28-row bins
    n_big = cfg.NC * (12 * 18 + 3)  # 1752
    heap = [(0, 0, 0, 0, b) for b in range(n_big)]
    heapq.heapify(heap)
    assign = [[] for _ in range(n_big)]
    loads = [(0, 0)] * n_big
    d0l = deg0[big_rows]
    d1l = deg1[big_rows]
    for i in range(len(big_rows)):
        r = big_rows[i]
        _, l0, l1, n, b = heapq.heappop(heap)
        assign[b].append(r)
        l0 += int(d0l[i])
        l1 += int(d1l[i])
        n += 1
        loads[b] = (l0, l1)
        if n < 28:
            heapq.heappush(heap, (max(l0, l1), l0, l1, n, b))

    # heavy bins grouped 8-at-a-time onto the same (win, slot) across cores
    bin_order = sorted(range(n_big), key=lambda b: -max(*loads[b]))
    big_positions = [(w, s) for w in range(12) for s in range(18)] + \
                    [(12, s) for s in range(3)]
    bins = {}
    for g, (w, s) in enumerate(big_positions):
        for c in range(cfg.NC):
            bins[(c, w, s)] = assign[bin_order[g * cfg.NC + c]]

    # small bins: lowest-degree rows, sequential fill
    p = 0
    for i in range(n_small8):
        c, w = i % cfg.NC, i // cfg.NC
        bins[(c, w, 18)] = list(small_rows[p:p + 8])
        p += 8
    for c in range(cfg.NC):
        bins[(c, 12, 3)] = list(small_rows[p:p + 22])
        p += 22

    perm = np.empty(cfg.N_T, np.int64)
    for c in range(cfg.NC):
        base = c * cfg.NT_LOC
        for w in range(cfg.n_wins):
            for s, (wd, roff) in enumerate(cfg.grid[w]):
                rows = bins[(c, w, s)]
                assert len(rows) == wd
                perm[base + w * cfg.WIN + roff:
                     base + w * cfg.WIN + roff + wd] = rows
    return perm


def _edge_fields(cfg, rows_new):
    """Map permuted row index -> (core, win, gslot, local_d)."""
    core = rows_new // cfg.NT_LOC
    lrow = rows_new % cfg.NT_LOC
    win = lrow // cfg.WIN
    wrow = lrow % cfg.WIN
    n28 = np.where(win < 12, 18, 3)
    s_idx = np.minimum(wrow // cfg.SW, n28)
    roff = np.where(s_idx < n28, s_idx * cfg.SW, n28 * cfg.SW)
    local_d = wrow - roff
    gslot = cfg.gslot_base[win] + s_idx
    return core, win, gslot, local_d


def _structure(cfg, k):
    """Derive chunk-stream layout from per-gslot chunk counts k (len 232).

    Returns dict with per-window chunk descriptor lists and bases."""
    k = np.asarray(k, np.int64)
    st = {"k": k}
    chunk_base = np.zeros(cfg.n_gslots, np.int64)
    np.cumsum(k[:-1], out=chunk_base[1:])
    st["chunk_base"] = chunk_base
    st["K_total"] = int(k.sum())
    win_chunks = []     # per window: list of (psc_off, width, start, stop)
    win_base = []       # first chunk col of window
    n28_l, ksm_l, smw_l, swidth_l = [], [], [], []
    for w in range(cfg.n_wins):
        g0 = cfg.gslot_base[w]
        chunks = []
        scol = 0
        n28 = 0
        for s, (wd, roff) in enumerate(cfg.grid[w]):
            kk = int(k[g0 + s])
            for i in range(kk):
                chunks.append((roff, wd, i == 0, i == kk - 1))
            if wd == cfg.SW:
                n28 += kk
            else:
                ksm_l.append(kk)
                smw_l.append(wd)
            scol += kk * wd
        win_chunks.append(chunks)
        win_base.append(int(chunk_base[g0]))
        n28_l.append(n28)
        swidth_l.append(scol)
    st["win_chunks"] = win_chunks
    st["win_base"] = win_base
    st["n28"] = n28_l
    st["ksm"] = ksm_l          # per window small-slot chunk count
    st["smw"] = smw_l          # per window small-slot width (8 or 22)
    st["s_width"] = swidth_l   # one-hot tile cols per window
    st["Kwin_max"] = max(len(c) for c in win_chunks)
    st["swidth_max"] = max(swidth_l)
    st["n28_max"] = max(n28_l)
    st["k8_max"] = max(ksm_l[w] for w in range(12))
    st["k22_max"] = ksm_l[12]
    return st


def prep_inputs(cfg, inputs):
    deg0 = np.bincount(np.asarray(inputs["rows0"], np.int64), minlength=cfg.N_T)
    deg1 = np.bincount(np.asarray(inputs["rows1"], np.int64), minlength=cfg.N_T)
    perm = _balance(cfg, deg0, deg1)
    pos = np.empty(cfg.N_T, np.int64)
    pos[perm] = np.arange(cfg.N_T)

    # per-branch edge fields + per-gslot counts (max over cores+branches)
    br_fields = []
    counts = np.zeros((2, cfg.NC, cfg.n_gslots), np.int64)
    for b in (0, 1):
        rows_new = pos[np.asarray(inputs[f"rows{b}"], np.int64)]
        core, win, gslot, local_d = _edge_fields(cfg, rows_new)
        np.add.at(counts[b], (core, gslot), 1)
        br_fields.append((core, gslot, local_d))
    k = np.maximum(1, -(-counts.max(axis=(0, 1)) // 128))
    st = _structure(cfg, k)
    K = st["K_total"]
    chunk_base = st["chunk_base"]

    # pack x / d streams
    xd = {}
    for b in (0, 1):
        core, gslot, local_d = br_fields[b]
        cols = np.asarray(inputs[f"cols{b}"], np.int64)
        vals = np.asarray(inputs[f"vals{b}"], np.float32)
        xsrc = np.asarray(inputs[f"x_src{b}"], np.float32)
        key = core * cfg.n_gslots + gslot
        order = np.argsort(key, kind="stable")
        key_s = key[order]
        grp_counts = np.bincount(key_s, minlength=cfg.NC * cfg.n_gslots)
        starts = np.zeros(cfg.NC * cfg.n_gslots, np.int64)
        np.cumsum(grp_counts[:-1], out=starts[1:])
        rank = np.arange(len(key)) - starts[key_s]
        core_s = core[order]
        chunkcol = chunk_base[gslot[order]] + rank // 128
        lane = rank % 128
        x_arr = np.zeros((cfg.NC, 128, K, 128), BF16_NP)
        d_arr = np.zeros((cfg.NC, 128, K), BF16_NP)
        x_arr[core_s, lane, chunkcol] = \
            (vals[order][:, None] * xsrc[cols[order]]).astype(BF16_NP)
        d_arr[core_s, lane, chunkcol] = local_d[order].astype(BF16_NP)
        xd[f"x{b}"] = x_arr
        xd[f"d{b}"] = d_arr

    zero_bias = (not np.any(np.asarray(inputs["mlp_b0"]))
                 and not np.any(np.asarray(inputs["mlp_b1"])))

    # ---- weights (same layouts as v12)
    x_target = np.asarray(inputs["x_target"], np.float32)
    W0 = np.asarray(inputs["W0"], np.float32)
    W1 = np.asarray(inputs["W1"], np.float32)
    w01 = np.ascontiguousarray(np.concatenate([W0, W1], axis=1)).astype(BF16_NP)

    mlpw = []
    for b in (0, 1):
        mw = np.asarray(inputs[f"mlp_W{b}"], np.float32)
        blocks = []
        for l in range(cfg.N_MLP):
            for icb in range(2):
                for ocb in range(2):
                    blocks.append(mw[l, icb * 128:(icb + 1) * 128,
                                     ocb * 128:(ocb + 1) * 128])
        mlpw.append(np.ascontiguousarray(
            np.concatenate(blocks, axis=1)).astype(BF16_NP))

    mlpb = []
    for b in (0, 1):
        mb_ = np.asarray(inputs[f"mlp_b{b}"], np.float32)
        cols_ = []
        for l in range(cfg.N_MLP):
            for ocb in range(2):
                cols_.append(mb_[l, ocb * 128:(ocb + 1) * 128][:, None])
        mlpb.append(np.ascontiguousarray(np.concatenate(cols_, axis=1)))

    Wm = np.asarray(inputs["Wm"], np.float32)
    wm = np.ascontiguousarray(
        np.concatenate([Wm[i * 128:(i + 1) * 128, :] for i in range(4)], axis=1)
    ).astype(BF16_NP)
    bmt = np.ascontiguousarray(
        np.tile(np.asarray(inputs["bm"], np.float32), (128, 2)))

    in_maps = []
    for c in range(cfg.NC):
        xt_loc = x_target[perm[c * cfg.NT_LOC:(c + 1) * cfg.NT_LOC]]
        xt = np.zeros((128, cfg.n_wins * 2 * cfg.WIN), BF16_NP)
        for wi in range(cfg.n_wins):
            w0 = wi * cfg.WIN
            wl = min(cfg.WIN, cfg.NT_LOC - w0)
            for sub in range(2):
                s0 = sub * 256
                if s0 >= wl:
                    break
                sl = min(256, wl - s0)
                for ocb in range(2):
                    base = wi * 2 * cfg.WIN + sub * 512 + ocb * 256
                    xt[:, base:base + sl] = \
                        xt_loc[w0 + s0:w0 + s0 + sl,
                               ocb * 128:(ocb + 1) * 128].T
        m = {
            "xt": xt,
            "w01": w01, "mlpw0": mlpw[0], "mlpw1": mlpw[1],
            "b0": mlpb[0], "b1": mlpb[1],
            "wm": wm, "bmt": bmt,
        }
        for b in (0, 1):
            m[f"x{b}"] = np.ascontiguousarray(xd[f"x{b}"][c]).reshape(128, K * 128)
            m[f"d{b}"] = np.ascontiguousarray(xd[f"d{b}"][c])
        in_maps.append(m)
    return in_maps, st, zero_bias, perm


# ------------------------------------------------------------------- builder

def build(cfg, st, zero_bias):
    nc = bacc.Bacc("TRN2", target_bir_lowering=False, debug=False)

    K = st["K_total"]
    n_wins = cfg.n_wins
    x_d = [nc.declare_dram_parameter(f"x{b}", [128, K * 128], BF16, isOutput=False)
           for b in (0, 1)]
    d_d = [nc.declare_dram_parameter(f"d{b}", [128, K], BF16, isOutput=False)
           for b in (0, 1)]
    xt_d = nc.declare_dram_parameter("xt", [128, n_wins * 2 * cfg.WIN], BF16,
                                     isOutput=False)
    w01_d = nc.declare_dram_parameter("w01", [128, 512], BF16, isOutput=False)
    mlpw_d = [nc.declare_dram_parameter(f"mlpw{b}", [128, cfg.N_MLP * 4 * 128], BF16,
                                        isOutput=False) for b in (0, 1)]
    b_d = [nc.declare_dram_parameter(f"b{b}", [128, cfg.N_MLP * 2], F32, isOutput=False)
           for b in (0, 1)]
    wm_d = nc.declare_dram_parameter("wm", [128, 4 * cfg.OUT_CH], BF16, isOutput=False)
    bmt_d = nc.declare_dram_parameter("bmt", [128, 2 * cfg.OUT_CH], F32, isOutput=False)
    out_d = nc.declare_dram_parameter("out", [cfg.NT_LOC, cfg.OUT_CH], BF16,
                                      isOutput=True)

    wins = []
    w0 = 0
    while w0 < cfg.NT_LOC:
        wins.append((w0, min(cfg.WIN, cfg.NT_LOC - w0)))
        w0 += cfg.WIN

    RELU = mybir.ActivationFunctionType.Relu
    EQ = mybir.AluOpType.is_equal

    with tile.TileContext(nc) as tc:
        with (
            tc.tile_pool(name="wpool", bufs=1) as wpool,
            tc.tile_pool(name="xwin", bufs=3) as xpool,
            tc.tile_pool(name="xtwin", bufs=3) as xtpool,
            tc.tile_pool(name="swin", bufs=2) as spool,
            tc.tile_pool(name="aggp", bufs=2) as aggp,
            tc.tile_pool(name="hwin", bufs=2) as hwin,
            tc.tile_pool(name="hfin", bufs=2) as hfin,
            tc.tile_pool(name="outp", bufs=2) as outp,
            tc.tile_pool(name="pscat", bufs=2, space="PSUM") as pscat,
            tc.tile_pool(name="pdense", bufs=4, space="PSUM") as pdense,
            tc.tile_pool(name="pmerge", bufs=2, space="PSUM") as pmerge,
        ):
            # --- per-edge index streams first: the prologue scatter needs
            # only d + the first window's x, so keep big weight DMAs behind.
            d_sb = []
            for b in (0, 1):
                td = wpool.tile([128, K], BF16, tag=f"d{b}")
                nc.sync.dma_start(out=td[:], in_=d_d[b][:])
                d_sb.append(td)

            # --- iota tiles for on-device one-hot build
            io28 = wpool.tile([128, st["n28_max"] * 28], BF16, tag="io28")
            nc.gpsimd.iota(out=io28[:], pattern=[[0, st["n28_max"]], [1, 28]],
                           base=0, channel_multiplier=0,
                           allow_small_or_imprecise_dtypes=True)
            io8 = wpool.tile([128, st["k8_max"] * 8], BF16, tag="io8")
            nc.gpsimd.iota(out=io8[:], pattern=[[0, st["k8_max"]], [1, 8]],
                           base=0, channel_multiplier=0,
                           allow_small_or_imprecise_dtypes=True)
            io22 = wpool.tile([128, st["k22_max"] * 22], BF16, tag="io22")
            nc.gpsimd.iota(out=io22[:], pattern=[[0, st["k22_max"]], [1, 22]],
                           base=0, channel_multiplier=0,
                           allow_small_or_imprecise_dtypes=True)

            def load_win(wi):
                """Prefetch window wi's x chunk streams (both branches, split
                into two half-DMAs so scatter can start on the first half)
                and its x_target slice."""
                wb = st["win_base"][wi]
                kw = len(st["win_chunks"][wi])
                half = kw // 2
                tiles = []
                for br in (0, 1):
                    xw = xpool.tile([128, st["Kwin_max"] * 128], BF16, tag=f"xw{br}")
                    xh = [xw[:, :half * 128], xw[:, half * 128:kw * 128]]
                    nc.sync.dma_start(
                        out=xh[0],
                        in_=x_d[br][:, wb * 128:(wb + half) * 128])
                    nc.sync.dma_start(
                        out=xh[1],
                        in_=x_d[br][:, (wb + half) * 128:(wb + kw) * 128])
                    tiles.append(xw)
                xt_w = xtpool.tile([128, 2 * cfg.WIN], BF16, tag="xt")
                nc.sync.dma_start(
                    out=xt_w[:],
                    in_=xt_d[:, wi * 2 * cfg.WIN:(wi + 1) * 2 * cfg.WIN])
                tiles.append(xt_w)
                return tiles

            def emit_onehot(wi, br):
                """Build window wi's one-hot scatter tile for branch br."""
                wb = st["win_base"][wi]
                n28 = st["n28"][wi]
                ksm = st["ksm"][wi]
                smw = st["smw"][wi]
                s = spool.tile([128, st["swidth_max"]], BF16, tag=f"s{br}")
                nc.vector.tensor_tensor(
                    out=s[:, :n28 * 28].rearrange("p (n w) -> p n w", w=28),
                    in0=io28[:, :n28 * 28].rearrange("p (n w) -> p n w", w=28),
                    in1=d_sb[br][:, wb:wb + n28].unsqueeze(2)
                        .broadcast_to([128, n28, 28]),
                    op=EQ)
                off = n28 * 28
                iot = io8 if smw == 8 else io22
                nc.vector.tensor_tensor(
                    out=s[:, off:off + ksm * smw]
                        .rearrange("p (n w) -> p n w", w=smw),
                    in0=iot[:, :ksm * smw].rearrange("p (n w) -> p n w", w=smw),
                    in1=d_sb[br][:, wb + n28:wb + n28 + ksm].unsqueeze(2)
                        .broadcast_to([128, ksm, smw]),
                    op=EQ)
                return s

            # process the short (106-row) window FIRST: its x stream is tiny
            # (fast pipeline fill while weights stream in) and the final
            # window then has 4 full dependency chains (no 2-chain tail).
            worder = [len(wins) - 1] + list(range(len(wins) - 1))
            win_tiles = {worder[0]: load_win(worder[0])}

            # weights ride the scalar queue so they never block the x stream
            w01_sb = wpool.tile([128, 512], BF16, tag="w01")
            nc.scalar.dma_start(out=w01_sb[:], in_=w01_d[:])
            win_tiles[worder[1]] = load_win(worder[1])
            mlpw_sb, b_sb = [], []
            for b in (0, 1):
                t = wpool.tile([128, cfg.N_MLP * 4 * 128], BF16, tag=f"mlpw{b}")
                nc.scalar.dma_start(out=t[:], in_=mlpw_d[b][:])
                mlpw_sb.append(t)
                if not zero_bias:
                    tb = wpool.tile([128, cfg.N_MLP * 2], F32, tag=f"bias{b}")
                    nc.scalar.dma_start(out=tb[:], in_=b_d[b][:])
                    b_sb.append(tb)
            wm_sb = wpool.tile([128, 4 * cfg.OUT_CH], BF16, tag="wm")
            nc.scalar.dma_start(out=wm_sb[:], in_=wm_d[:])
            bmt_sb = wpool.tile([128, 2 * cfg.OUT_CH], F32, tag="bmt")
            nc.scalar.dma_start(out=bmt_sb[:], in_=bmt_d[:])

            hfin_store = {}
            sc_state = {}
            aggw_store = {}
            out_rr = [0]

            def emit_sc_half(wi, br, half):
                """Emit one half of window wi / branch br's scatter matmuls;
                on the second half, copy the PSUM row-block to bf16 SBUF."""
                if wi is None:
                    return
                w0, wl = wins[wi]
                chunks = st["win_chunks"][wi]
                total = len(chunks)
                lo, hi = (0, total // 2) if half == 0 else (total // 2, total)
                if half == 0:
                    sc_state[(wi, br)] = (
                        pscat.tile([128, cfg.WIN], F32, tag="psc",
                                   name=f"psc{wi}_{br}"),
                        emit_onehot(wi, br),
                    )
                psc, s = sc_state[(wi, br)]
                xw = win_tiles[wi][br]
                scol = sum(c[1] for c in chunks[:lo])
                for idx in range(lo, hi):
                    off, wd, sta, sto = chunks[idx]
                    nc.tensor.matmul(
                        out=psc[:, off:off + wd],
                        lhsT=xw[:, idx * 128:(idx + 1) * 128],
                        rhs=s[:, scol:scol + wd],
                        start=sta, stop=sto)
                    scol += wd
                if half == 1:
                    ag = aggp.tile([128, cfg.WIN], BF16, tag=f"agg{br}")
                    nc.vector.tensor_copy(out=ag[:, :wl], in_=psc[:, :wl])
                    aggw_store[(wi, br)] = ag
                    del sc_state[(wi, br)]

            def emit_merge(wi, cur=None, tps=None):
                """Merge window wi: concat(h0,h1) @ Wm + bm -> out rows."""
                w0, wl = wins[wi]
                nb = -(-wl // 128)
                curw = hfin_store.pop(wi) if cur is None else cur
                for tp in (range(0, nb, 2) if tps is None else tps):
                    seg_n = min(2, nb - tp)
                    po = pmerge.tile([128, cfg.WIN], F32, tag="po")
                    for s in range(seg_n):
                        r0 = (tp + s) * 128
                        rl = min(128, wl - r0)
                        si = r0 // 256
                        rr = r0 - si * 256
                        for ic in range(4):
                            nc.tensor.matmul(
                                out=po[:rl, s * 256:s * 256 + cfg.OUT_CH],
                                lhsT=curw[(ic // 2, si)][:, (ic % 2) * 256 + rr:
                                                         (ic % 2) * 256 + rr + rl],
                                rhs=wm_sb[:, ic * cfg.OUT_CH:(ic + 1) * cfg.OUT_CH],
                                start=(ic == 0), stop=(ic == 3))
                    o_sb = outp.tile([128, 2 * cfg.OUT_CH], BF16, tag="o")
                    width = seg_n * 256
                    nc.vector.tensor_tensor(
                        out=o_sb[:, :width], in0=po[:, :width],
                        in1=bmt_sb[:, :width], op=mybir.AluOpType.add)
                    for s in range(seg_n):
                        r0 = (tp + s) * 128
                        rl = min(128, wl - r0)
                        # round-robin output writes across three queues so the
                        # tail flush isn't serialized on one engine
                        eng = (nc.gpsimd, nc.scalar)[out_rr[0] % 2]
                        out_rr[0] += 1
                        eng.dma_start(
                            out=out_d[w0 + r0:w0 + r0 + rl, :],
                            in_=o_sb[:rl, s * 256:s * 256 + 256])

            # prologue: scatter the first window fully
            for br in (0, 1):
                emit_sc_half(worder[0], br, 0)
                emit_sc_half(worder[0], br, 1)

            for pos, wi in enumerate(worder):
                w0, wl = wins[wi]
                last_pos = pos == len(worder) - 1
                nxt = worder[pos + 1] if pos + 1 < len(worder) else None
                if pos + 2 < len(worder):
                    win_tiles[worder[pos + 2]] = load_win(worder[pos + 2])
                aggw = [aggw_store.pop((wi, 0)), aggw_store.pop((wi, 1))]
                xt_w = win_tiles[wi][2]

                subs = [(0, min(256, wl))]
                if wl > 256:
                    subs.append((256, wl - 256))
                chains = [(br, si) for br in (0, 1) for si in range(len(subs))]
                cur = {}
                for br, si in chains:
                    s0, sl = subs[si]
                    ph = pdense.tile([128, cfg.WIN], F32, tag="pd",
                                     name=f"ph{br}{si}")
                    for ocb in range(2):
                        nc.tensor.matmul(
                            out=ph[:, ocb * 256:ocb * 256 + sl],
                            lhsT=w01_sb[:, br * 256 + ocb * 128:
                                        br * 256 + ocb * 128 + 128],
                            rhs=aggw[br][:, s0:s0 + sl],
                            start=True, stop=True)
                    h = hwin.tile([128, cfg.WIN], BF16, tag=f"h{br}{si}",
                                  name=f"h{br}{si}")
                    nc.vector.tensor_tensor(
                        out=h[:], in0=ph[:],
                        in1=xt_w[:, si * 512:si * 512 + 512],
                        op=mybir.AluOpType.add)
                    cur[(br, si)] = h
                emit_sc_half(nxt, 0, 0)
                for l in range(cfg.N_MLP):
                    last = l == cfg.N_MLP - 1
                    for br, si in chains:
                        s0, sl = subs[si]
                        pm = pdense.tile([128, cfg.WIN], F32, tag="pd",
                                         name=f"pm{l}{br}{si}")
                        for ocb in range(2):
                            for icb in range(2):
                                nc.tensor.matmul(
                                    out=pm[:, ocb * 256:ocb * 256 + sl],
                                    lhsT=mlpw_sb[br][:, (l * 4 + icb * 2 + ocb) * 128:
                                                     (l * 4 + icb * 2 + ocb) * 128 + 128],
                                    rhs=cur[(br, si)][:, icb * 256:icb * 256 + sl],
                                    start=(icb == 0), stop=(icb == 1))
                        pool = hfin if last else hwin
                        hn = pool.tile([128, cfg.WIN], BF16,
                                       tag=(f"hf{br}{si}" if last
                                            else f"h{br}{si}"),
                                       name=f"hn{l}{br}{si}")
                        if zero_bias:
                            nc.scalar.activation(out=hn[:], in_=pm[:],
                                                 func=RELU, bias=0.0)
                        else:
                            for ocb in range(2):
                                nc.scalar.activation(
                                    out=hn[:, ocb * 256:(ocb + 1) * 256],
                                    in_=pm[:, ocb * 256:(ocb + 1) * 256],
                                    func=RELU,
                                    bias=b_sb[br][:, l * 2 + ocb:l * 2 + ocb + 1])
                        cur[(br, si)] = hn
                    if l == 0:
                        emit_sc_half(nxt, 0, 1)
                        emit_sc_half(nxt, 1, 0)
                    elif l == 1:
                        emit_sc_half(nxt, 1, 1)
                hfin_store[wi] = cur
                if pos > 0:
                    emit_merge(worder[pos - 1])

            emit_merge(worder[-1])

    nc.compile()
    return nc


# -------------------------------------------------------------------- runner

_CACHE = {}


def kernel(**inputs) -> np.ndarray:
    _install_profile_hook()
    cfg = CFG
    in_maps, st, zero_bias, perm = prep_inputs(cfg, inputs)
    key = ("v13", tuple(int(v) for v in st["k"]), zero_bias)
    if key not in _CACHE:
        _CACHE[key] = build(cfg, st, zero_bias)
    nc = _CACHE[key]
    trace = bool(int(__import__("os").environ.get("KERNEL_TRACE", "0")))
    r = run_bass_kernel_spmd(nc, in_maps, core_ids=list(range(cfg.NC)), trace=trace)
    kernel.last_result = r
    res = np.concatenate([r.results[c]["out"] for c in range(cfg.NC)], axis=0)
    out = np.empty((cfg.N_T, cfg.OUT_CH), np.float32)
    out[perm] = res
    return out


kernel.last_result = None
